# revision 6
# baseline (speedup 1.0000x reference)
"""CrossProductLayer kernel for Trainium2 (Bass/Tile), 8-core data parallel.

out[b, :] = concat(x[b]**2, x[b], 0.5 * x[b,i]*x[b,j] for i<j) * w

Full inputs:  x [16384, 128] f32, w [8384] f32.
Full output:  [16384, 8384] f32.

Sharding: pure data parallel on the batch dim — each of the 8 cores gets
2048 rows of x; w (pre-scaled by the 0.5 pair factor and pre-broadcast to
[128, 8384] on host) is replicated. Forward only, no collectives.

Per-core device kernel, column-panel architecture:
  All 16 row-tiles (128 batch rows each) are processed together. The 8384
  output columns are split into ~8 panels; a panel super-tile in SBUF is
  [128 partitions, 16 tiles x panel_cols]. Each pair block i
  (out[:, blk_i] = x[:,i] * x[:,i+1:]) lies in one panel and is computed
  for all 16 tiles at once with a single broadcast-AP tensor_tensor op
  (VectorE or GpSimdE), or per-tile activation ops on ScalarE (which can
  exploit its per-partition scale operand but cannot group tiles). The *w
  multiply is one or two in-place grouped tensor_tensor ops per panel
  (VectorE/GpSimdE split). Blocks are assigned to engines by a greedy
  balance using trace-calibrated per-op costs. VectorE issues only
  tensor_tensor ops (1-port) so GpSimdE never contends for the shared
  SBUF port. One HWDGE DMA stores each panel (~8.6 MB).
"""

import numpy as np

B = 16384
NI = 128
NF = NI + NI + (NI * (NI - 1)) // 2  # 8384
NCORES = 8
ROWS = B // NCORES  # 2048
TILE_P = 128
TILES = ROWS // TILE_P  # 16
PAIRS_OFF = 2 * NI  # 256

PANEL_CAP = 1072  # max output columns per panel

# trace-calibrated per-op cost model (ns), used only for engine balancing
ACT_OVH_CYC = 335.0  # ScalarE per-op overhead (cycles @ 1.2 GHz)
DVE_OVH_CYC = 140.0  # VectorE per-op overhead (cycles @ 0.96 GHz)
GP_CYC_PER_EL = 2.6  # GpSimd 2-input cycles per element per lane @ 1.2 GHz
GP_OVH_NS = 250.0


def _plan():
    """Split columns into panels; assign each pair block to an engine."""
    widths = [NI - 1 - i for i in range(NI - 1)]
    starts = []
    off = PAIRS_OFF
    for w in widths:
        starts.append(off)
        off += w
    assert off == NF

    panels = []  # (c0, cols, head: bool, blocks)
    c0, cols, head, blocks = 0, PAIRS_OFF, True, []
    for i in range(NI - 1):
        w = widths[i]
        if cols + w > PANEL_CAP:
            panels.append((c0, cols, head, blocks))
            c0, cols, head, blocks = starts[i], 0, False, []
        blocks.append(i)
        cols += w
    panels.append((c0, cols, head, blocks))

    def cost_a(w):
        return TILES * (ACT_OVH_CYC + w) / 1.2

    def cost_g(w):
        return TILES * w * GP_CYC_PER_EL / 1.2 + GP_OVH_NS

    def cost_d(w):
        return TILES * w / 0.96 + DVE_OVH_CYC / 0.96

    dve_ns = TILES / 0.96  # *w pass, per col
    gp_ns = TILES * GP_CYC_PER_EL / 1.2

    plan = []
    for c0, cols, head, blocks in panels:
        by_w = sorted(blocks, key=lambda i: -widths[i])
        a_head = TILES * (ACT_OVH_CYC + NI) / 1.2 if head else 0.0
        best = None
        for n_a in range(len(blocks) + 1):
            a_load = a_head + sum(cost_a(widths[i]) for i in by_w[:n_a])
            rest = by_w[n_a:]
            # continuous D/G estimate: D rate 1/dve_ns, G rate 1/gp_ns
            c_rem = sum(widths[i] for i in rest) + cols
            t_dg = c_rem * dve_ns * gp_ns / (dve_ns + gp_ns)
            mk = max(a_load, t_dg)
            if best is None or mk < best[0]:
                best = (mk, n_a)
        n_a = best[1]
        assign = {}
        load = {"A": a_head, "G": 0.0, "D": 0.0}
        for i in by_w[:n_a]:
            assign[i] = "A"
            load["A"] += cost_a(widths[i])
        for i in by_w[n_a:]:
            # assign to whichever of D/G finishes it sooner
            if load["D"] + cost_d(widths[i]) <= load["G"] + cost_g(widths[i]):
                assign[i] = "D"
                load["D"] += cost_d(widths[i])
            else:
                assign[i] = "G"
                load["G"] += cost_g(widths[i])
        # split the *w pass columns between DVE and GpSimd to balance
        x = (load["G"] - load["D"] + gp_ns * cols) / (dve_ns + gp_ns)
        wd = int(np.clip(round(x), 0, cols))
        load["D"] += wd * dve_ns
        load["G"] += (cols - wd) * gp_ns
        plan.append(
            {
                "c0": c0,
                "cols": cols,
                "head": head,
                "blocks": blocks,
                "assign": assign,
                "w_dve_cols": wd,
                "load": load,
            }
        )
    return widths, starts, plan


WIDTHS, STARTS, PLAN = _plan()

_CACHE = {}


def _build_nc():
    from concourse import bacc
    import concourse.mybir as mybir
    from concourse.tile import TileContext

    f32 = mybir.dt.float32
    nc = bacc.Bacc(
        "TRN2",
        target_bir_lowering=False,
        debug=False,
        num_devices=NCORES,
    )
    x_d = nc.dram_tensor("x", [ROWS, NI], f32, kind="ExternalInput")
    w_d = nc.dram_tensor("w", [NI, NF], f32, kind="ExternalInput")
    o_d = nc.dram_tensor("out", [ROWS, NF], f32, kind="ExternalOutput")

    x_hbm3 = x_d.rearrange("(t p) c -> p t c", t=TILES)  # [128, 16, 128]
    o_hbm3 = o_d.rearrange("(t p) c -> p t c", t=TILES)  # [128, 16, 8384]

    pmax = max(p["cols"] for p in PLAN)

    with TileContext(nc) as tc:
        with (
            tc.tile_pool(name="xp", bufs=1) as xp,
            tc.tile_pool(name="wp", bufs=1) as wp,
            tc.tile_pool(name="pp", bufs=2) as pp,
        ):
            x_all = xp.tile([TILE_P, TILES * NI], f32)
            x3 = x_all[:].rearrange("p (t c) -> p t c", t=TILES)
            nc.sync.dma_start(out=x3, in_=x_hbm3)
            w_t = wp.tile([NI, NF], f32)
            nc.sync.dma_start(out=w_t[:], in_=w_d[:])

            for p in PLAN:
                c0, cols, head = p["c0"], p["cols"], p["head"]
                pan = pp.tile([TILE_P, TILES * pmax], f32, tag="pan")
                pan3 = pan[:, : TILES * cols].rearrange(
                    "p (t c) -> p t c", t=TILES
                )
                if head:
                    # singles straight from HBM; squares on ScalarE
                    nc.sync.dma_start(
                        out=pan3[:, :, NI : 2 * NI], in_=x_hbm3
                    )
                    nc.scalar.square(pan3[:, :, 0:NI], x3)
                for i in p["blocks"]:
                    w = WIDTHS[i]
                    c = STARTS[i] - c0
                    eng = p["assign"][i]
                    src = x3[:, :, i + 1 : i + 1 + w]
                    dst = pan3[:, :, c : c + w]
                    if eng == "A":
                        for t in range(TILES):
                            nc.scalar.mul(
                                dst[:, t], src[:, t], x3[:, t, i : i + 1]
                            )
                    else:
                        bcast = x3[:, :, i : i + 1].broadcast_to(
                            [TILE_P, TILES, w]
                        )
                        if eng == "D":
                            nc.vector.tensor_mul(dst, bcast, src)
                        else:
                            nc.gpsimd.tensor_mul(dst, bcast, src)
                # in-place *w pass, split between VectorE and GpSimdE
                wd = p["w_dve_cols"]
                wsl = w_t[:, None, c0 : c0 + cols]
                if wd > 0:
                    nc.vector.tensor_mul(
                        pan3[:, :, 0:wd],
                        pan3[:, :, 0:wd],
                        wsl[:, :, 0:wd].broadcast_to([TILE_P, TILES, wd]),
                    )
                if wd < cols:
                    nc.gpsimd.tensor_mul(
                        pan3[:, :, wd:cols],
                        pan3[:, :, wd:cols],
                        wsl[:, :, wd:cols].broadcast_to(
                            [TILE_P, TILES, cols - wd]
                        ),
                    )
                nc.sync.dma_start(
                    out=o_hbm3[:, :, c0 : c0 + cols], in_=pan3
                )
    nc.compile()
    return nc


def _get_nc():
    if "nc" not in _CACHE:
        _CACHE["nc"] = _build_nc()
    return _CACHE["nc"]


def _prep_in_maps(x, w):
    x = np.ascontiguousarray(np.asarray(x, dtype=np.float32))
    w = np.asarray(w, dtype=np.float32)
    w_scaled = w.copy()
    w_scaled[PAIRS_OFF:] *= np.float32(0.5)
    w_b = np.ascontiguousarray(np.broadcast_to(w_scaled[None, :], (NI, NF)))
    return [
        {"x": np.ascontiguousarray(x[c * ROWS : (c + 1) * ROWS]), "w": w_b}
        for c in range(NCORES)
    ]


def _run(x, w, trace=False, tmpdir=None):
    from concourse.bass_utils import run_bass_kernel_spmd

    nc = _get_nc()
    in_maps = _prep_in_maps(x, w)
    res = run_bass_kernel_spmd(
        nc, in_maps, list(range(NCORES)), trace=trace, tmpdir=tmpdir
    )
    out = np.concatenate([res.results[c]["out"] for c in range(NCORES)], axis=0)
    return out, res


def kernel(**inputs):
    out, _ = _run(inputs["x"], inputs["w"])
    return out


if __name__ == "__main__":
    for p in PLAN:
        na = sum(1 for e in p["assign"].values() if e == "A")
        ng = sum(1 for e in p["assign"].values() if e == "G")
        nd = sum(1 for e in p["assign"].values() if e == "D")
        print(
            f"panel c0={p['c0']:5d} cols={p['cols']:5d} head={int(p['head'])} "
            f"blocks={len(p['blocks']):3d} A/G/D={na}/{ng}/{nd} "
            f"w_dve={p['w_dve_cols']:4d} "
            f"load A={p['load']['A']/1e3:6.1f}us G={p['load']['G']/1e3:6.1f}us "
            f"D={p['load']['D']/1e3:6.1f}us"
        )
    tot = {e: sum(p["load"][e] for p in PLAN) for e in "AGD"}
    print({k: f"{v/1e3:.1f}us" for k, v in tot.items()})


# revision 8
# speedup vs baseline: 1.0781x; 1.0781x over previous
"""CrossProductLayer kernel for Trainium2 (Bass/Tile), 8-core data parallel.

out[b, :] = concat(x[b]**2, x[b], 0.5 * x[b,i]*x[b,j] for i<j) * w

Full inputs:  x [16384, 128] f32, w [8384] f32.
Full output:  [16384, 8384] f32.

Sharding: pure data parallel on the batch dim — each of the 8 cores gets
2048 rows of x; w (pre-scaled by the 0.5 pair factor and pre-broadcast to
[128, 8384] on host) is replicated. Forward only, no collectives.

Per-core device kernel: all 16 row-tiles (128 batch rows) are processed
together; a pair block i (out[:, blk_i] = x[:,i] * x[:,i+1:]) is one
grouped op over a [128, 16, w] AP. The 127 blocks are split into three
contiguous stripes, one per engine, so the three engines run decoupled
pipelines with no per-panel lockstep:
  - ScalarE:  widest blocks, per-tile activation ops (per-partition scale
              amortizes its high per-op cost); squares + singles head.
              Its *w pass is done by VectorE.
  - GpSimdE:  middle blocks, grouped broadcast tensor_tensor + its own
              in-place *w pass (self-contained pipeline).
  - VectorE:  narrow tail blocks + *w passes for itself and ScalarE.
VectorE issues only tensor_tensor ops (1-port) so GpSimdE never contends
for the shared SBUF port. Each stripe is chunked (~440 output cols) into
a bufs=2 pool; every chunk ends in one HWDGE DMA store (~3.6 MB).
"""

import numpy as np

B = 16384
NI = 128
NF = NI + NI + (NI * (NI - 1)) // 2  # 8384
NCORES = 8
ROWS = B // NCORES  # 2048
TILE_P = 128
TILES = ROWS // TILE_P  # 16
PAIRS_OFF = 2 * NI  # 256

CHUNK_COLS = 440  # max output columns per chunk tile

# calibrated cost model (ns), only used to pick the stripe boundaries
ACT_NS = lambda w: TILES * (445.0 + w) / 1.2
DVE_COL = 1.05 / 0.96 * TILES  # ns per output col (one pass)
GP_COL = 2.4 / 1.2 * TILES

WIDTHS = [NI - 1 - i for i in range(NI - 1)]
STARTS = []
_off = PAIRS_OFF
for _w in WIDTHS:
    STARTS.append(_off)
    _off += _w
assert _off == NF


def _plan():
    """Pick stripe boundaries A (ACT) and B (GpSimd) to balance engines."""
    best = None
    for A in range(10, 60):
        act = TILES * (445 + NI) / 1.2  # squares
        act += sum(ACT_NS(WIDTHS[i]) for i in range(A))
        wa = sum(WIDTHS[:A])
        rest = 8128 - wa
        # DVE does W for head+ACT stripe (256+wa cols) + its own stripe
        # (pairs + W); GP does pairs + W for its stripe.
        # split rest = g + d_pairs, DVE = (256+wa)*DVE_COL + d*2*DVE_COL,
        # GP = g*2*GP_COL; balance.
        fixed_d = (256 + wa) * DVE_COL
        g = (fixed_d + 2 * DVE_COL * rest) / (2 * GP_COL + 2 * DVE_COL)
        g = min(max(g, 0.0), rest)
        gp = g * 2 * GP_COL
        dve = fixed_d + (rest - g) * 2 * DVE_COL
        mk = max(act, gp, dve)
        if best is None or mk < best[0]:
            best = (mk, A, g)
    _, A, g = best
    # B: first index so that sum of widths in [A, B) >= g
    acc, Bb = 0.0, A
    while Bb < NI - 1 and acc < g:
        acc += WIDTHS[Bb]
        Bb += 1
    return A, Bb


STRIPE_A, STRIPE_B = _plan()


def _chunks(i0, i1, head=False):
    """Split blocks [i0, i1) into chunks of <= CHUNK_COLS output cols.
    Returns list of (c0, cols, blocks, head)."""
    out = []
    c0 = 0 if head else STARTS[i0]
    cols = PAIRS_OFF if head else 0
    blocks = []
    for i in range(i0, i1):
        w = WIDTHS[i]
        if cols + w > CHUNK_COLS and cols > 0:
            out.append((c0, cols, blocks, head))
            c0, cols, blocks, head = STARTS[i], 0, [], False
        blocks.append(i)
        cols += w
    if cols > 0:
        out.append((c0, cols, blocks, head))
    return out


_CACHE = {}


def _build_nc():
    from concourse import bacc
    import concourse.mybir as mybir
    from concourse.tile import TileContext

    f32 = mybir.dt.float32
    MUL = mybir.AluOpType.mult
    nc = bacc.Bacc(
        "TRN2",
        target_bir_lowering=False,
        debug=False,
        num_devices=NCORES,
    )
    x_d = nc.dram_tensor("x", [ROWS, NI], f32, kind="ExternalInput")
    w_d = nc.dram_tensor("w", [NI, NF], f32, kind="ExternalInput")
    o_d = nc.dram_tensor("out", [ROWS, NF], f32, kind="ExternalOutput")

    x_hbm3 = x_d.rearrange("(t p) c -> p t c", t=TILES)  # [128, 16, 128]
    o_hbm3 = o_d.rearrange("(t p) c -> p t c", t=TILES)  # [128, 16, 8384]

    a_chunks = _chunks(0, STRIPE_A, head=True)
    g_chunks = _chunks(STRIPE_A, STRIPE_B)
    d_chunks = _chunks(STRIPE_B, NI - 1)

    def pairs_op(eng, pan3, c0, i):
        w = WIDTHS[i]
        c = STARTS[i] - c0
        src = x3[:, :, i + 1 : i + 1 + w]
        dst = pan3[:, :, c : c + w]
        bc = x3[:, :, i : i + 1].broadcast_to([TILE_P, TILES, w])
        eng.tensor_mul(dst, bc, src)

    def w_op(eng, pan3, c0, cols):
        wbc = w_t[:, None, c0 : c0 + cols].broadcast_to(
            [TILE_P, TILES, cols]
        )
        eng.tensor_mul(pan3, pan3, wbc)

    with TileContext(nc) as tc:
        with (
            tc.tile_pool(name="xp", bufs=1) as xp,
            tc.tile_pool(name="wp", bufs=1) as wp,
            tc.tile_pool(name="ap", bufs=2) as apool,
            tc.tile_pool(name="gp", bufs=2) as gpool,
            tc.tile_pool(name="dp", bufs=2) as dpool,
        ):
            x_all = xp.tile([TILE_P, TILES * NI], f32)
            x3 = x_all[:].rearrange("p (t c) -> p t c", t=TILES)
            nc.sync.dma_start(out=x3, in_=x_hbm3)
            w_t = wp.tile([NI, NF], f32)
            nc.sync.dma_start(out=w_t[:], in_=w_d[:])

            # interleave chunk issue round-robin across the three stripes
            # so program order roughly matches time order
            seqs = [
                ("A", apool, a_chunks),
                ("G", gpool, g_chunks),
                ("D", dpool, d_chunks),
            ]
            maxlen = max(len(c) for _, _, c in seqs)
            for k in range(maxlen):
                for eng_name, pool, chunks in seqs:
                    if k >= len(chunks):
                        continue
                    c0, cols, blocks, head = chunks[k]
                    pan = pool.tile(
                        [TILE_P, TILES * CHUNK_COLS],
                        f32,
                        name=f"pan{eng_name}{k}",
                        tag=f"pan{eng_name}",
                    )
                    pan3 = pan[:, : TILES * cols].rearrange(
                        "p (t c) -> p t c", t=TILES
                    )
                    if head:
                        nc.sync.dma_start(
                            out=pan3[:, :, NI : 2 * NI], in_=x_hbm3
                        )
                        nc.scalar.square(pan3[:, :, 0:NI], x3)
                    for i in blocks:
                        if eng_name == "A":
                            w = WIDTHS[i]
                            c = STARTS[i] - c0
                            for t in range(TILES):
                                nc.scalar.mul(
                                    pan3[:, t, c : c + w],
                                    x3[:, t, i + 1 : i + 1 + w],
                                    x3[:, t, i : i + 1],
                                )
                        elif eng_name == "G":
                            pairs_op(nc.gpsimd, pan3, c0, i)
                        else:
                            pairs_op(nc.vector, pan3, c0, i)
                    # *w pass: DVE covers its own and ACT's chunks
                    w_op(nc.gpsimd if eng_name == "G" else nc.vector,
                         pan3, c0, cols)
                    nc.sync.dma_start(
                        out=o_hbm3[:, :, c0 : c0 + cols], in_=pan3
                    )
    nc.compile()
    return nc


def _get_nc():
    if "nc" not in _CACHE:
        _CACHE["nc"] = _build_nc()
    return _CACHE["nc"]


def _prep_in_maps(x, w):
    x = np.ascontiguousarray(np.asarray(x, dtype=np.float32))
    w = np.asarray(w, dtype=np.float32)
    w_scaled = w.copy()
    w_scaled[PAIRS_OFF:] *= np.float32(0.5)
    w_b = np.ascontiguousarray(np.broadcast_to(w_scaled[None, :], (NI, NF)))
    return [
        {"x": np.ascontiguousarray(x[c * ROWS : (c + 1) * ROWS]), "w": w_b}
        for c in range(NCORES)
    ]


def _run(x, w, trace=False, tmpdir=None):
    from concourse.bass_utils import run_bass_kernel_spmd

    nc = _get_nc()
    in_maps = _prep_in_maps(x, w)
    res = run_bass_kernel_spmd(
        nc, in_maps, list(range(NCORES)), trace=trace, tmpdir=tmpdir
    )
    out = np.concatenate([res.results[c]["out"] for c in range(NCORES)], axis=0)
    return out, res


def kernel(**inputs):
    out, _ = _run(inputs["x"], inputs["w"])
    return out


if __name__ == "__main__":
    print("A =", STRIPE_A, "B =", STRIPE_B)
    wa = sum(WIDTHS[: STRIPE_A])
    wg = sum(WIDTHS[STRIPE_A:STRIPE_B])
    wd = sum(WIDTHS[STRIPE_B:])
    act = TILES * (445 + NI) / 1.2 + sum(ACT_NS(WIDTHS[i]) for i in range(STRIPE_A))
    gp = wg * 2 * GP_COL
    dve = (256 + wa) * DVE_COL + wd * 2 * DVE_COL
    print(f"cols A/G/D = {wa}/{wg}/{wd}")
    print(f"pred ACT={act/1e3:.0f}us GP={gp/1e3:.0f}us DVE={dve/1e3:.0f}us")
    for nm, ch in [("A", _chunks(0, STRIPE_A, True)),
                   ("G", _chunks(STRIPE_A, STRIPE_B)),
                   ("D", _chunks(STRIPE_B, NI - 1))]:
        print(nm, [(c0, cols, len(b)) for c0, cols, b, _ in ch])
